# revision 4
# baseline (speedup 1.0000x reference)
"""Trainium2 Bass kernel for nn_DampedIMEX1Layer (v2).

Math: the per-step 2x2 transition M (per diagonal state p) is constant over
time, so the associative scan is a constant-coefficient linear recurrence.
Per core (= one batch element, data-parallel over 8 cores), chunk T=8 over
L=8192 (C=1024 chunks):

  local    out[:, i, tau] += sum_{s<=tau} Phi_s x[:, i, tau-s]   (PE, fp16)
  extract  hhat streams (comp k, re/im) with V^{-1}-folded weights (PE)
  chain    per-p normal form M^8 = V rR(th) V^{-1}: twiddle by unit
           rotations (DVE TT), hardware prefix scan with real per-partition
           multiplier r (tensor_tensor_scan), untwiddle+shift into S' (DVE)
  inject   out[:, i, tau] += Psi[tau,k,ri] S'[k,ri] accumulated into the
           same PSUM groups as local (PE) -- no assembly pass
  out      tau-major on device; host unshuffles (free wrt HW time)

All matmuls fp16 (PE 1 cycle/row, pipelined LDWEIGHTS + FWL); chain
elementwise ops fp16 (DVE 2x mode), scan state fp32 internal.
Defective (clamped-A) p's use a Jordan form: kappa cross-term via one STT,
twiddle rows are identity. Validated in numpy (proto.py): rel err ~6e-3.
"""
import numpy as np

import concourse.bacc as bacc
import concourse.mybir as mybir
from concourse.tile import TileContext
from concourse import bass_utils

P = 128
H = 128
L = 8192
BSZ = 8
T = 8
C = L // T              # 1024 chunks
SEG = 2
CS = C // SEG           # 512 chunks per segment

F32 = mybir.dt.float32
F16 = mybir.dt.float16


def _host_params(A_diag, G_diag, dt):
    f = np.float32
    dt_s = (1.0 / (1.0 + np.exp(-dt.astype(np.float64)))).astype(f)
    A = np.maximum(A_diag.astype(f), f(0.0))
    G = np.maximum(G_diag.astype(f), f(0.0))
    dt2 = np.maximum(dt_s * dt_s, f(1e-6))
    s = np.sqrt(f(1.0) + dt_s * G)
    A_low = (f(2.0) + dt_s * G - f(2.0) * s) / dt2
    A_high = (f(2.0) + dt_s * G + f(2.0) * s) / dt2
    A_fin = A_low + np.maximum(A - A_low, f(0)) - np.maximum(A - A_high, f(0))
    S = f(1.0) + dt_s * G
    M11 = f(1.0) / S
    M12 = -(dt_s / S) * A_fin
    M21 = dt_s / S
    M22 = f(1.0) - (dt_s * dt_s / S) * A_fin
    c1 = dt_s / S
    c2 = dt_s * dt_s / S
    M = np.stack([np.stack([M11, M12], -1), np.stack([M21, M22], -1)], -2)
    c = np.stack([c1, c2], -1)
    return M.astype(np.float64), c.astype(np.float64)


def _normal_form(M):
    """Per-p real normal form M = V K V^{-1} with K = r R(th) or Jordan."""
    Pn = M.shape[0]
    V = np.zeros((Pn, 2, 2))
    r = np.zeros(Pn)
    th = np.zeros(Pn)
    kap = np.zeros(Pn)
    for p in range(Pn):
        a, b = M[p, 0, 0], M[p, 0, 1]
        cc, d = M[p, 1, 0], M[p, 1, 1]
        m = 0.5 * (a + d)
        disc = (a - d) ** 2 + 4 * b * cc
        if disc < -1e-12:
            w = 0.5 * np.sqrt(-disc)
            lam = m + 1j * w
            u = np.array([b, lam - a]) if abs(b) > 1e-14 else np.array([lam - d, cc])
            # balance |Re u| vs |Im u| with a complex phase (keeps rotation form)
            phi = 0.5 * np.angle(u @ u)
            u = u * np.exp(-1j * phi)
            ur, ui = u.real, u.imag
            Vp = np.stack([ur, -ui], axis=1)
            Vp /= np.sqrt(max(np.linalg.norm(ur) * np.linalg.norm(ui), 1e-30))
            V[p] = Vp
            r[p] = np.hypot(m, w)
            th[p] = np.arctan2(w, m)
        else:
            N = M[p] - m * np.eye(2)
            r[p] = m
            if np.linalg.norm(N) < 1e-12:
                V[p] = np.eye(2)
            else:
                j = int(np.argmax(np.linalg.norm(N, axis=0)))
                v2 = np.eye(2)[:, j]
                v1 = N @ v2
                v1 /= np.linalg.norm(v1)
                V[p] = np.stack([v1, v2], axis=1)
            K = np.linalg.solve(V[p], M[p] @ V[p])
            kap[p] = K[0, 1]
        K = np.linalg.solve(V[p], M[p] @ V[p])
        if disc < -1e-12:
            Kx = r[p] * np.array(
                [[np.cos(th[p]), -np.sin(th[p])], [np.sin(th[p]), np.cos(th[p])]])
        else:
            Kx = np.array([[r[p], kap[p]], [0, r[p]]])
        assert np.allclose(K, Kx, atol=2e-6), (p, K, Kx)
    return V, np.linalg.inv(V), r, th, kap


def _host_weights(A_diag, G_diag, dt, B, C_, D):
    M1, c = _host_params(A_diag, G_diag, dt)
    V, Vinv, r1, th1, kap1 = _normal_form(M1)
    rc = r1 ** T
    thc = th1 * T
    kapc = T * r1 ** (T - 1) * kap1      # (rI+kN)^T = r^T I + T r^{T-1} kN

    Bre = B[..., 0].astype(np.float64)
    Bim = B[..., 1].astype(np.float64)
    Cre = C_[..., 0].astype(np.float64)
    Cim = C_[..., 1].astype(np.float64)

    Mp = [np.tile(np.eye(2), (P, 1, 1))]
    for _ in range(T):
        Mp.append(np.einsum('pij,pjk->pik', M1, Mp[-1]))

    # local Phi lag matrices; lhsT = Phi.T, slot s at cols s*H
    K_s = np.stack([np.einsum('pij,pj->pi', Mp[s], c)[:, 1] for s in range(T)])
    wphi = np.empty((H, T * H), np.float16)
    for s in range(T):
        Phi = (Cre * K_s[s]) @ Bre - (Cim * K_s[s]) @ Bim
        if s == 0:
            Phi = Phi + np.diag(D.astype(np.float64))
        wphi[:, s * H:(s + 1) * H] = Phi.T.astype(np.float16)

    # extraction: q_j = Vinv M^{7-j} c; stream q=k*2+ri; lhsT (H,P) = W.T
    qj = np.stack([np.einsum('pij,pj->pi', Vinv @ Mp[T - 1 - j], c)
                   for j in range(T)])  # (T,P,2)
    wv = np.empty((H, T * 4 * P), np.float16)
    for j in range(T):
        for k in range(2):
            for ri, Bx in ((0, Bre), (1, Bim)):
                W = Bx * qj[j, :, k][:, None]         # (P,H)
                sl = (j * 4 + k * 2 + ri) * P
                wv[:, sl:sl + P] = W.T.astype(np.float16)

    # injection: w_tau_k = (M^{tau+1} V)[1,k]; lhsT (P,H) = Psi.T
    wpsi = np.empty((P, T * 4 * H), np.float16)
    for tau in range(T):
        wtk = np.einsum('pij,pjk->pik', Mp[tau + 1], V)[:, 1, :]  # (P,2)
        for k in range(2):
            for ri, Cx, sgn in ((0, Cre, 1.0), (1, Cim, -1.0)):
                Psi = sgn * Cx * wtk[:, k]            # (H,P)
                sl = (tau * 4 + k * 2 + ri) * H
                wpsi[:, sl:sl + H] = Psi.T.astype(np.float16)

    ii = np.arange(C)
    twc = np.cos(thc[:, None] * ii[None, :]).astype(np.float16)
    tws = np.sin(thc[:, None] * ii[None, :]).astype(np.float16)
    rcb = np.tile(rc[:, None].astype(np.float32), (1, CS))        # (P,512) f32
    kapv = np.ascontiguousarray(kapc[:, None].astype(np.float32))  # (P,1)

    return dict(wphi=np.ascontiguousarray(wphi), wv=np.ascontiguousarray(wv),
                wpsi=np.ascontiguousarray(wpsi),
                twc=np.ascontiguousarray(twc), tws=np.ascontiguousarray(tws),
                rcb=np.ascontiguousarray(rcb), kapv=kapv)


def _build_nc():
    nc = bacc.Bacc("TRN2", target_bir_lowering=False, debug=False, num_devices=8)
    Alu = mybir.AluOpType

    x_d = nc.dram_tensor("x", (H, L), F16, kind="ExternalInput").ap()
    wv_d = nc.dram_tensor("wv", (H, T * 4 * P), F16, kind="ExternalInput").ap()
    wphi_d = nc.dram_tensor("wphi", (H, T * H), F16, kind="ExternalInput").ap()
    wpsi_d = nc.dram_tensor("wpsi", (P, T * 4 * H), F16, kind="ExternalInput").ap()
    twc_d = nc.dram_tensor("twc", (P, C), F16, kind="ExternalInput").ap()
    tws_d = nc.dram_tensor("tws", (P, C), F16, kind="ExternalInput").ap()
    rcb_d = nc.dram_tensor("rcb", (P, CS), F32, kind="ExternalInput").ap()
    kap_d = nc.dram_tensor("kapv", (P, 1), F32, kind="ExternalInput").ap()
    out_d = nc.dram_tensor("out", (H, L), F32, kind="ExternalOutput").ap()

    with TileContext(nc) as tc:
        with (
            tc.tile_pool(name="const", bufs=1) as cp,
            tc.tile_pool(name="tmp", bufs=1) as tp,
            tc.tile_pool(name="ps", bufs=1, space="PSUM") as pp,
        ):
            # ---------- loads ----------
            x_sb = cp.tile([H, L], F16, tag="x")
            for sg in range(SEG):
                nc.sync.dma_start(x_sb[:, sg * T * CS:(sg + 1) * T * CS],
                                  x_d[:, sg * T * CS:(sg + 1) * T * CS])
            wv_sb = cp.tile([H, T * 4 * P], F16, tag="wv")
            nc.sync.dma_start(wv_sb[:], wv_d)
            wphi_sb = cp.tile([H, T * H], F16, tag="wphi")
            nc.sync.dma_start(wphi_sb[:], wphi_d)
            wpsi_sb = cp.tile([P, T * 4 * H], F16, tag="wpsi")
            nc.sync.dma_start(wpsi_sb[:], wpsi_d)
            twc = cp.tile([P, C], F16, tag="twc")
            nc.sync.dma_start(twc[:], twc_d)
            tws = cp.tile([P, C], F16, tag="tws")
            nc.sync.dma_start(tws[:], tws_d)
            rcb = cp.tile([P, CS], F32, tag="rcb")
            nc.sync.dma_start(rcb[:], rcb_d)
            kap = cp.tile([P, 1], F32, tag="kap")
            nc.sync.dma_start(kap[:], kap_d)

            x3 = x_sb[:].rearrange("p (c t) -> p c t", t=T)

            # ---------- extraction (PE) ----------
            # stream q = k*2+ri; psum tags ps0..ps7 = (seg*4+q)
            hh = [cp.tile([P, C], F16, tag=f"hh{q}", name=f"hh{q}")
                  for q in range(4)]
            for sg in range(SEG):
                pse = [pp.tile([P, CS], F32, tag=f"ps{sg * 4 + q}",
                                    name=f"pse{sg}{q}") for q in range(4)]
                for j in range(T):
                    for q in range(4):
                        sl = (j * 4 + q) * P
                        nc.tensor.matmul(
                            pse[q][:], wv_sb[:, sl:sl + P],
                            x3[:, sg * CS:(sg + 1) * CS, j],
                            start=(j == 0), stop=(j == T - 1))
                for q in range(4):
                    nc.scalar.copy(hh[q][:, sg * CS:(sg + 1) * CS], pse[q][:])

            # ---------- chain (DVE) ----------
            # S'[k][ri][:, i] = chi_k^{ri}(i-1); col 0 = 0
            sp = [[cp.tile([P, C], F16, tag=f"sp{k}{ri}", name=f"sp{k}{ri}")
                   for ri in range(2)] for k in range(2)]
            v1p = tp.tile([P, C + 1], F16, tag="v1p")
            v2p = tp.tile([P, C + 1], F16, tag="v2p")
            m1 = tp.tile([P, CS], F16, tag="m1")
            m2 = tp.tile([P, CS], F16, tag="m2")
            vin1 = tp.tile([P, CS], F16, tag="vin1")
            vin2 = tp.tile([P, CS], F16, tag="vin2")
            vin1c = tp.tile([P, CS], F16, tag="vin1c")
            for k in range(2):
                for ri in range(2):
                    nc.vector.memset(sp[k][ri][:, 0:1].bitcast(F16), 0.0)
            nc.vector.memset(v2p[:, 0:1].bitcast(F16), 0.0)

            for ri in range(2):
                h1 = hh[0 * 2 + ri]
                h2 = hh[1 * 2 + ri]
                for sg in range(SEG):
                    sl = slice(sg * CS, (sg + 1) * CS)
                    sl1 = slice(sg * CS + 1, (sg + 1) * CS + 1)
                    # vin2 = twc*h2 - tws*h1
                    nc.vector.tensor_tensor(m1[:], twc[:, sl], h2[:, sl], Alu.mult)
                    nc.vector.tensor_tensor(m2[:], tws[:, sl], h1[:, sl], Alu.mult)
                    nc.vector.tensor_tensor(vin2[:], m1[:], m2[:], Alu.subtract)
                    # v2(i) = rc v2(i-1) + vin2(i)  [state fp32 internal]
                    init2 = 0.0 if sg == 0 else v2p[:, sg * CS:sg * CS + 1]
                    nc.vector.tensor_tensor_scan(
                        v2p[:, sl1], rcb[:], vin2[:], init2, Alu.mult, Alu.add)
                    # vin1 = twc*h1 + tws*h2 (+ kap * v2(i-1) Jordan cross)
                    nc.vector.tensor_tensor(m1[:], twc[:, sl], h1[:, sl], Alu.mult)
                    nc.vector.tensor_tensor(m2[:], tws[:, sl], h2[:, sl], Alu.mult)
                    nc.vector.tensor_tensor(vin1[:], m1[:], m2[:], Alu.add)
                    nc.vector.scalar_tensor_tensor(
                        vin1c[:], v2p[:, sl], kap[:, 0:1], vin1[:],
                        Alu.mult, Alu.add)
                    init1 = 0.0 if sg == 0 else v1p[:, sg * CS:sg * CS + 1]
                    nc.vector.tensor_tensor_scan(
                        v1p[:, sl1], rcb[:], vin1c[:], init1, Alu.mult, Alu.add)
                    # untwiddle + shift: S'(i) = R(+th(i-1)) v(i-1)
                    # dst cols [sg*CS+1 : (sg+1)*CS+1) clipped to < C
                    n = CS if sg < SEG - 1 else CS - 1
                    d = slice(sg * CS + 1, sg * CS + 1 + n)
                    tbl = slice(sg * CS, sg * CS + n)          # twc(i-1)
                    vsl = slice(sg * CS + 1, sg * CS + 1 + n)  # v1p[i] = v(i-1)
                    nc.vector.tensor_tensor(m1[:, :n], twc[:, tbl], v1p[:, vsl],
                                            Alu.mult)
                    nc.vector.tensor_tensor(m2[:, :n], tws[:, tbl], v2p[:, vsl],
                                            Alu.mult)
                    nc.vector.tensor_tensor(sp[0][ri][:, d], m1[:, :n], m2[:, :n],
                                            Alu.subtract)
                    nc.vector.tensor_tensor(m1[:, :n], tws[:, tbl], v1p[:, vsl],
                                            Alu.mult)
                    nc.vector.tensor_tensor(m2[:, :n], twc[:, tbl], v2p[:, vsl],
                                            Alu.mult)
                    nc.vector.tensor_tensor(sp[1][ri][:, d], m1[:, :n], m2[:, :n],
                                            Alu.add)

            # ---------- local + inject (PE), tau-major out ----------
            out_sb = cp.tile([H, L], F32, tag="out")
            for sg in range(SEG):
                psl = [pp.tile([P, CS], F32, tag=f"ps{tau}", name=f"psl{sg}{tau}")
                       for tau in range(T)]
                for s in range(T):
                    for tau in range(s, T):
                        nc.tensor.matmul(
                            psl[tau][:], wphi_sb[:, s * H:(s + 1) * H],
                            x3[:, sg * CS:(sg + 1) * CS, tau - s],
                            start=(s == 0), stop=False)
                for tau in range(T):
                    for q in range(4):
                        sl = (tau * 4 + q) * H
                        nc.tensor.matmul(
                            psl[tau][:], wpsi_sb[:, sl:sl + H],
                            sp[q // 2][q % 2][:, sg * CS:(sg + 1) * CS],
                            start=False, stop=(q == 3))
                    dst = out_sb[:, tau * C + sg * CS: tau * C + (sg + 1) * CS]
                    nc.scalar.copy(dst, psl[tau][:])
                    nc.sync.dma_start(
                        out_d[:, tau * C + sg * CS: tau * C + (sg + 1) * CS], dst)

    nc.compile()
    return nc


_NC_CACHE = None


def _prep(inputs):
    x = np.asarray(inputs["x"], np.float32)
    wts = _host_weights(
        np.asarray(inputs["A_diag"], np.float32),
        np.asarray(inputs["G_diag"], np.float32),
        np.asarray(inputs["dt"], np.float32),
        np.asarray(inputs["B"], np.float32),
        np.asarray(inputs["C"], np.float32),
        np.asarray(inputs["D"], np.float32))
    xt = np.ascontiguousarray(x.transpose(0, 2, 1)).astype(np.float16)
    return [dict(wts, x=xt[b]) for b in range(BSZ)]


def kernel(x, A_diag, G_diag, dt, B, C, D):
    global _NC_CACHE
    if _NC_CACHE is None:
        _NC_CACHE = _build_nc()
    in_maps = _prep(dict(x=x, A_diag=A_diag, G_diag=G_diag, dt=dt, B=B, C=C, D=D))
    res = bass_utils.run_bass_kernel_spmd(
        _NC_CACHE, in_maps, core_ids=list(range(BSZ)), trace=False)
    out = np.stack([res.results[b]["out"] for b in range(BSZ)], 0)  # (B,H,L) tau-major
    out = out.reshape(BSZ, H, T, L // T).transpose(0, 3, 2, 1).reshape(BSZ, L, H)
    return np.ascontiguousarray(out)


# revision 6
# speedup vs baseline: 1.8602x; 1.8602x over previous
"""Trainium2 Bass kernel for nn_DampedIMEX1Layer (v2).

Math: the per-step 2x2 transition M (per diagonal state p) is constant over
time, so the associative scan is a constant-coefficient linear recurrence.
Per core (= one batch element, data-parallel over 8 cores), chunk T=8 over
L=8192 (C=1024 chunks):

  local    out[:, i, tau] += sum_{s<=tau} Phi_s x[:, i, tau-s]   (PE, fp16)
  extract  hhat streams (comp k, re/im) with V^{-1}-folded weights (PE)
  chain    per-p normal form M^8 = V rR(th) V^{-1}: twiddle by unit
           rotations (DVE TT), hardware prefix scan with real per-partition
           multiplier r (tensor_tensor_scan), untwiddle+shift into S' (DVE)
  inject   out[:, i, tau] += Psi[tau,k,ri] S'[k,ri] accumulated into the
           same PSUM groups as local (PE) -- no assembly pass
  out      tau-major on device; host unshuffles (free wrt HW time)

All matmuls fp16 (PE 1 cycle/row, pipelined LDWEIGHTS + FWL); chain
elementwise ops fp16 (DVE 2x mode), scan state fp32 internal.
Defective (clamped-A) p's use a Jordan form: kappa cross-term via one STT,
twiddle rows are identity. Validated in numpy (proto.py): rel err ~6e-3.
"""
import numpy as np

import concourse.bacc as bacc
import concourse.mybir as mybir
from concourse.tile import TileContext
from concourse import bass_utils

P = 128
H = 128
L = 8192
BSZ = 8
T = 8
C = L // T              # 1024 chunks
SEG = 2
CS = C // SEG           # 512 chunks per segment

F32 = mybir.dt.float32
F16 = mybir.dt.float16


def _host_params(A_diag, G_diag, dt):
    f = np.float32
    dt_s = (1.0 / (1.0 + np.exp(-dt.astype(np.float64)))).astype(f)
    A = np.maximum(A_diag.astype(f), f(0.0))
    G = np.maximum(G_diag.astype(f), f(0.0))
    dt2 = np.maximum(dt_s * dt_s, f(1e-6))
    s = np.sqrt(f(1.0) + dt_s * G)
    A_low = (f(2.0) + dt_s * G - f(2.0) * s) / dt2
    A_high = (f(2.0) + dt_s * G + f(2.0) * s) / dt2
    A_fin = A_low + np.maximum(A - A_low, f(0)) - np.maximum(A - A_high, f(0))
    S = f(1.0) + dt_s * G
    M11 = f(1.0) / S
    M12 = -(dt_s / S) * A_fin
    M21 = dt_s / S
    M22 = f(1.0) - (dt_s * dt_s / S) * A_fin
    c1 = dt_s / S
    c2 = dt_s * dt_s / S
    M = np.stack([np.stack([M11, M12], -1), np.stack([M21, M22], -1)], -2)
    c = np.stack([c1, c2], -1)
    return M.astype(np.float64), c.astype(np.float64)


def _normal_form(M):
    """Per-p real normal form M = V K V^{-1} with K = r R(th) or Jordan."""
    Pn = M.shape[0]
    V = np.zeros((Pn, 2, 2))
    r = np.zeros(Pn)
    th = np.zeros(Pn)
    kap = np.zeros(Pn)
    for p in range(Pn):
        a, b = M[p, 0, 0], M[p, 0, 1]
        cc, d = M[p, 1, 0], M[p, 1, 1]
        m = 0.5 * (a + d)
        disc = (a - d) ** 2 + 4 * b * cc
        if disc < -1e-12:
            w = 0.5 * np.sqrt(-disc)
            lam = m + 1j * w
            u = np.array([b, lam - a]) if abs(b) > 1e-14 else np.array([lam - d, cc])
            # balance |Re u| vs |Im u| with a complex phase (keeps rotation form)
            phi = 0.5 * np.angle(u @ u)
            u = u * np.exp(-1j * phi)
            ur, ui = u.real, u.imag
            Vp = np.stack([ur, -ui], axis=1)
            Vp /= np.sqrt(max(np.linalg.norm(ur) * np.linalg.norm(ui), 1e-30))
            V[p] = Vp
            r[p] = np.hypot(m, w)
            th[p] = np.arctan2(w, m)
        else:
            N = M[p] - m * np.eye(2)
            r[p] = m
            if np.linalg.norm(N) < 1e-12:
                V[p] = np.eye(2)
            else:
                j = int(np.argmax(np.linalg.norm(N, axis=0)))
                v2 = np.eye(2)[:, j]
                v1 = N @ v2
                v1 /= np.linalg.norm(v1)
                V[p] = np.stack([v1, v2], axis=1)
            K = np.linalg.solve(V[p], M[p] @ V[p])
            kap[p] = K[0, 1]
        K = np.linalg.solve(V[p], M[p] @ V[p])
        if disc < -1e-12:
            Kx = r[p] * np.array(
                [[np.cos(th[p]), -np.sin(th[p])], [np.sin(th[p]), np.cos(th[p])]])
        else:
            Kx = np.array([[r[p], kap[p]], [0, r[p]]])
        assert np.allclose(K, Kx, atol=2e-6), (p, K, Kx)
    return V, np.linalg.inv(V), r, th, kap


def _host_weights(A_diag, G_diag, dt, B, C_, D):
    M1, c = _host_params(A_diag, G_diag, dt)
    V, Vinv, r1, th1, kap1 = _normal_form(M1)
    rc = r1 ** T
    thc = th1 * T
    kapc = T * r1 ** (T - 1) * kap1      # (rI+kN)^T = r^T I + T r^{T-1} kN

    Bre = B[..., 0].astype(np.float64)
    Bim = B[..., 1].astype(np.float64)
    Cre = C_[..., 0].astype(np.float64)
    Cim = C_[..., 1].astype(np.float64)

    Mp = [np.tile(np.eye(2), (P, 1, 1))]
    for _ in range(T):
        Mp.append(np.einsum('pij,pjk->pik', M1, Mp[-1]))

    # local Phi lag matrices; lhsT = Phi.T, slot s at cols s*H
    K_s = np.stack([np.einsum('pij,pj->pi', Mp[s], c)[:, 1] for s in range(T)])
    wphi = np.empty((H, T * H), np.float16)
    for s in range(T):
        Phi = (Cre * K_s[s]) @ Bre - (Cim * K_s[s]) @ Bim
        if s == 0:
            Phi = Phi + np.diag(D.astype(np.float64))
        wphi[:, s * H:(s + 1) * H] = Phi.T.astype(np.float16)

    # extraction: q_j = Vinv M^{7-j} c; stream q=k*2+ri; lhsT (H,P) = W.T
    qj = np.stack([np.einsum('pij,pj->pi', Vinv @ Mp[T - 1 - j], c)
                   for j in range(T)])  # (T,P,2)
    wv = np.empty((H, T * 4 * P), np.float16)
    for j in range(T):
        for k in range(2):
            for ri, Bx in ((0, Bre), (1, Bim)):
                W = Bx * qj[j, :, k][:, None]         # (P,H)
                sl = (j * 4 + k * 2 + ri) * P
                wv[:, sl:sl + P] = W.T.astype(np.float16)

    # injection: w_tau_k = (M^{tau+1} V)[1,k]; lhsT (P,H) = Psi.T
    wpsi = np.empty((P, T * 4 * H), np.float16)
    for tau in range(T):
        wtk = np.einsum('pij,pjk->pik', Mp[tau + 1], V)[:, 1, :]  # (P,2)
        for k in range(2):
            for ri, Cx, sgn in ((0, Cre, 1.0), (1, Cim, -1.0)):
                Psi = sgn * Cx * wtk[:, k]            # (H,P)
                sl = (tau * 4 + k * 2 + ri) * H
                wpsi[:, sl:sl + H] = Psi.T.astype(np.float16)

    ii = np.arange(C)
    twc = np.cos(thc[:, None] * ii[None, :]).astype(np.float16)
    tws = np.sin(thc[:, None] * ii[None, :]).astype(np.float16)
    rcb = np.tile(rc[:, None].astype(np.float32), (1, CS))        # (P,512) f32
    kapv = np.ascontiguousarray(kapc[:, None].astype(np.float32))  # (P,1)

    return dict(wphi=np.ascontiguousarray(wphi), wv=np.ascontiguousarray(wv),
                wpsi=np.ascontiguousarray(wpsi),
                twc=np.ascontiguousarray(twc), tws=np.ascontiguousarray(tws),
                rcb=np.ascontiguousarray(rcb), kapv=kapv)


def _build_nc():
    nc = bacc.Bacc("TRN2", target_bir_lowering=False, debug=False, num_devices=8)
    Alu = mybir.AluOpType

    x_d = nc.dram_tensor("x", (H, L), F16, kind="ExternalInput").ap()  # tau-major
    wv_d = nc.dram_tensor("wv", (H, T * 4 * P), F16, kind="ExternalInput").ap()
    wphi_d = nc.dram_tensor("wphi", (H, T * H), F16, kind="ExternalInput").ap()
    wpsi_d = nc.dram_tensor("wpsi", (P, T * 4 * H), F16, kind="ExternalInput").ap()
    twc_d = nc.dram_tensor("twc", (P, C), F16, kind="ExternalInput").ap()
    tws_d = nc.dram_tensor("tws", (P, C), F16, kind="ExternalInput").ap()
    rcb_d = nc.dram_tensor("rcb", (P, CS), F32, kind="ExternalInput").ap()
    kap_d = nc.dram_tensor("kapv", (P, 1), F32, kind="ExternalInput").ap()
    out_d = nc.dram_tensor("out", (H, L), F32, kind="ExternalOutput").ap()

    with TileContext(nc) as tc:
        with (
            tc.tile_pool(name="const", bufs=1) as cp,
            tc.tile_pool(name="tmp", bufs=1) as tp,
            tc.tile_pool(name="ps", bufs=1, space="PSUM") as pp,
        ):
            # ---------- loads ----------
            x_sb = cp.tile([H, L], F16, tag="x")
            xv = x_sb[:].rearrange("p (t c) -> p t c", t=T)
            xdv = x_d.rearrange("p (t c) -> p t c", t=T)
            for sg in range(SEG):
                nc.sync.dma_start(xv[:, :, sg * CS:(sg + 1) * CS],
                                  xdv[:, :, sg * CS:(sg + 1) * CS])
            wv_sb = cp.tile([H, T * 4 * P], F16, tag="wv")
            nc.gpsimd.dma_start(wv_sb[:], wv_d)
            wphi_sb = cp.tile([H, T * H], F16, tag="wphi")
            nc.gpsimd.dma_start(wphi_sb[:], wphi_d)
            wpsi_sb = cp.tile([P, T * 4 * H], F16, tag="wpsi")
            nc.gpsimd.dma_start(wpsi_sb[:], wpsi_d)
            twc = cp.tile([P, C], F16, tag="twc")
            nc.scalar.dma_start(twc[:], twc_d)
            tws = cp.tile([P, C], F16, tag="tws")
            nc.scalar.dma_start(tws[:], tws_d)
            rcb = cp.tile([P, CS], F32, tag="rcb")
            nc.scalar.dma_start(rcb[:], rcb_d)
            kap = cp.tile([P, 1], F32, tag="kap")
            nc.scalar.dma_start(kap[:], kap_d)

            x3 = x_sb[:].rearrange("p (t c) -> p t c", t=T)

            # ---------- extraction (PE) ----------
            # stream q = k*2+ri; psum tags ps0..ps7 = (seg*4+q)
            hh = [cp.tile([P, C], F16, tag=f"hh{q}", name=f"hh{q}")
                  for q in range(4)]
            for sg in range(SEG):
                pse = [pp.tile([P, CS], F32, tag=f"ps{sg * 4 + q}",
                                    name=f"pse{sg}{q}") for q in range(4)]
                for j in range(T):
                    for q in range(4):
                        sl = (j * 4 + q) * P
                        nc.tensor.matmul(
                            pse[q][:], wv_sb[:, sl:sl + P],
                            x3[:, j, sg * CS:(sg + 1) * CS],
                            start=(j == 0), stop=(j == T - 1))
                for q in range(4):
                    nc.scalar.copy(hh[q][:, sg * CS:(sg + 1) * CS], pse[q][:])

            # ---------- chain (DVE) ----------
            # S'[k][ri][:, i] = chi_k^{ri}(i-1); col 0 = 0
            sp = [[cp.tile([P, C], F16, tag=f"sp{k}{ri}", name=f"sp{k}{ri}")
                   for ri in range(2)] for k in range(2)]
            v1p = tp.tile([P, C + 1], F16, tag="v1p")
            v2p = tp.tile([P, C + 1], F16, tag="v2p")
            m1 = tp.tile([P, CS], F16, tag="m1")
            m2 = tp.tile([P, CS], F16, tag="m2")
            vin1 = tp.tile([P, CS], F16, tag="vin1")
            vin2 = tp.tile([P, CS], F16, tag="vin2")
            vin1c = tp.tile([P, CS], F16, tag="vin1c")
            for k in range(2):
                for ri in range(2):
                    nc.vector.memset(sp[k][ri][:, 0:1].bitcast(F16), 0.0)
            nc.vector.memset(v2p[:, 0:1].bitcast(F16), 0.0)

            for ri in range(2):
                h1 = hh[0 * 2 + ri]
                h2 = hh[1 * 2 + ri]
                for sg in range(SEG):
                    sl = slice(sg * CS, (sg + 1) * CS)
                    sl1 = slice(sg * CS + 1, (sg + 1) * CS + 1)
                    # vin2 = twc*h2 - tws*h1
                    nc.vector.tensor_tensor(m1[:], twc[:, sl], h2[:, sl], Alu.mult)
                    nc.vector.tensor_tensor(m2[:], tws[:, sl], h1[:, sl], Alu.mult)
                    nc.vector.tensor_tensor(vin2[:], m1[:], m2[:], Alu.subtract)
                    # v2(i) = rc v2(i-1) + vin2(i)  [state fp32 internal]
                    init2 = 0.0 if sg == 0 else v2p[:, sg * CS:sg * CS + 1]
                    nc.vector.tensor_tensor_scan(
                        v2p[:, sl1], rcb[:], vin2[:], init2, Alu.mult, Alu.add)
                    # vin1 = twc*h1 + tws*h2 (+ kap * v2(i-1) Jordan cross)
                    nc.vector.tensor_tensor(m1[:], twc[:, sl], h1[:, sl], Alu.mult)
                    nc.vector.tensor_tensor(m2[:], tws[:, sl], h2[:, sl], Alu.mult)
                    nc.vector.tensor_tensor(vin1[:], m1[:], m2[:], Alu.add)
                    nc.vector.scalar_tensor_tensor(
                        vin1c[:], v2p[:, sl], kap[:, 0:1], vin1[:],
                        Alu.mult, Alu.add)
                    init1 = 0.0 if sg == 0 else v1p[:, sg * CS:sg * CS + 1]
                    nc.vector.tensor_tensor_scan(
                        v1p[:, sl1], rcb[:], vin1c[:], init1, Alu.mult, Alu.add)
                    # untwiddle + shift: S'(i) = R(+th(i-1)) v(i-1)
                    # dst cols [sg*CS+1 : (sg+1)*CS+1) clipped to < C
                    n = CS if sg < SEG - 1 else CS - 1
                    d = slice(sg * CS + 1, sg * CS + 1 + n)
                    tbl = slice(sg * CS, sg * CS + n)          # twc(i-1)
                    vsl = slice(sg * CS + 1, sg * CS + 1 + n)  # v1p[i] = v(i-1)
                    nc.vector.tensor_tensor(m1[:, :n], twc[:, tbl], v1p[:, vsl],
                                            Alu.mult)
                    nc.vector.tensor_tensor(m2[:, :n], tws[:, tbl], v2p[:, vsl],
                                            Alu.mult)
                    nc.vector.tensor_tensor(sp[0][ri][:, d], m1[:, :n], m2[:, :n],
                                            Alu.subtract)
                    nc.vector.tensor_tensor(m1[:, :n], tws[:, tbl], v1p[:, vsl],
                                            Alu.mult)
                    nc.vector.tensor_tensor(m2[:, :n], twc[:, tbl], v2p[:, vsl],
                                            Alu.mult)
                    nc.vector.tensor_tensor(sp[1][ri][:, d], m1[:, :n], m2[:, :n],
                                            Alu.add)

            # ---------- local + inject (PE), tau-major out ----------
            out_sb = cp.tile([H, L], F32, tag="out")
            for sg in range(SEG):
                psl = [pp.tile([P, CS], F32, tag=f"ps{tau}", name=f"psl{sg}{tau}")
                       for tau in range(T)]
                for s in range(T):
                    for tau in range(s, T):
                        nc.tensor.matmul(
                            psl[tau][:], wphi_sb[:, s * H:(s + 1) * H],
                            x3[:, tau - s, sg * CS:(sg + 1) * CS],
                            start=(s == 0), stop=False)
                for tau in range(T):
                    for q in range(4):
                        sl = (tau * 4 + q) * H
                        nc.tensor.matmul(
                            psl[tau][:], wpsi_sb[:, sl:sl + H],
                            sp[q // 2][q % 2][:, sg * CS:(sg + 1) * CS],
                            start=False, stop=(q == 3))
                    dst = out_sb[:, tau * C + sg * CS: tau * C + (sg + 1) * CS]
                    nc.scalar.copy(dst, psl[tau][:])
                    nc.sync.dma_start(
                        out_d[:, tau * C + sg * CS: tau * C + (sg + 1) * CS], dst)

    nc.compile()
    return nc


_NC_CACHE = None


def _prep(inputs):
    x = np.asarray(inputs["x"], np.float32)
    wts = _host_weights(
        np.asarray(inputs["A_diag"], np.float32),
        np.asarray(inputs["G_diag"], np.float32),
        np.asarray(inputs["dt"], np.float32),
        np.asarray(inputs["B"], np.float32),
        np.asarray(inputs["C"], np.float32),
        np.asarray(inputs["D"], np.float32))
    # (B,L,H) -> (B,H,T,C) tau-major flat (H, L)
    xt = x.reshape(BSZ, L // T, T, H).transpose(0, 3, 2, 1)
    xt = np.ascontiguousarray(xt.reshape(BSZ, H, L)).astype(np.float16)
    return [dict(wts, x=xt[b]) for b in range(BSZ)]


def kernel(x, A_diag, G_diag, dt, B, C, D):
    global _NC_CACHE
    if _NC_CACHE is None:
        _NC_CACHE = _build_nc()
    in_maps = _prep(dict(x=x, A_diag=A_diag, G_diag=G_diag, dt=dt, B=B, C=C, D=D))
    res = bass_utils.run_bass_kernel_spmd(
        _NC_CACHE, in_maps, core_ids=list(range(BSZ)), trace=False)
    out = np.stack([res.results[b]["out"] for b in range(BSZ)], 0)  # (B,H,L) tau-major
    out = out.reshape(BSZ, H, T, L // T).transpose(0, 3, 2, 1).reshape(BSZ, L, H)
    return np.ascontiguousarray(out)
